# revision 14
# baseline (speedup 1.0000x reference)
"""Trainium2 Bass kernel for the binarized-MLP (BNN) forward pass.

Strategy (data-parallel over batch, 8 cores x 512 rows):

  Layer 0 (fp32 x @ sign(W0).T, then batchnorm + sign):
    By linearity, h0 - mean_batch(h0) == (x - colmean(x)) @ sign(W0).T, so the
    batch mean never needs to be computed on device for layer 0 (no
    collective).  The centered x is quantized host-side to fixed point
    (scale 2^24) and split into three base-512 signed digit planes, each
    exactly representable in fp16.  Three fp16 matmuls accumulate EXACT small
    integers in fp32 PSUM (|plane sum| < 2^19 << 2^24, so no rounding ever),
    and a Horner recombine (acc = (P2*512 + P1)*512 + P0) on the vector
    engine reproduces the exact sign of h0 - mu0 for every element
    (host-verified: 0 flips, min margin 44 fixed-point units).
  Layers 1..3: activations and sign(W) are {-1,0,+1}; products and sums are
    small integers, so fp8(e4m3) DoubleRow matmuls with fp32 PSUM
    accumulation are EXACT.  Batch stats are integer column sums -> one small
    AllReduce per layer; mu = sum/4096 is exact in fp32.  Binarize =
    clamp((h-mu)*2^50, -1, 1), which reproduces sign() exactly including
    sign(0)=0 ties.
  Layer 3 output: out = (h3 - mu3) * rsqrt(var3 + eps) with var from an
    allreduced sum/sumsq and one Newton step after the reciprocal+sqrt.

The kernel() entry takes FULL unsharded inputs and returns the FULL output.
"""

import numpy as np
import ml_dtypes

import concourse.bass as bass
from concourse import bacc
import concourse.mybir as mybir
from concourse.tile import TileContext
from concourse.bass_utils import run_bass_kernel_spmd

P = 128
B = 512                # per-core batch (4096 / 8)
NCORES = 8
BATCH = 4096
EPS = 1e-5
SCALE = float(2 ** 24)  # fixed-point scale for centered x
BASE = 512.0            # digit base
BIG = float(2 ** 50)    # sign-extraction multiplier

K0, N0 = 2048, 4096     # layer 0
K1, N1 = 4096, 4096
K2, N2 = 4096, 4096
K3, N3 = 4096, 1024     # N3 padded 1000 -> 1024

F32 = mybir.dt.float32
F16 = mybir.dt.float16
FP8 = mybir.dt.float8e4
DR = mybir.MatmulPerfMode.DoubleRow

_CACHE = {}


def _build_nc():
    nc = bacc.Bacc(num_devices=NCORES)

    xd = [nc.declare_dram_parameter(f"xd{j}", [P, K0 // P, B], F16, isOutput=False)
          for j in range(3)]
    w0 = nc.declare_dram_parameter("w0", [N0 // P, P, K0 // P, P], F16, isOutput=False)
    w1 = nc.declare_dram_parameter("w1", [N1 // P, P, K1 // P, P], FP8, isOutput=False)
    w2 = nc.declare_dram_parameter("w2", [N2 // P, P, K2 // P, P], FP8, isOutput=False)
    w3 = nc.declare_dram_parameter("w3", [N3 // P, P, K3 // P, P], FP8, isOutput=False)
    out = nc.declare_dram_parameter("out", [N3 // P, P, B], F32, isOutput=True)

    # collective bounce buffers (internal DRAM; output must be Shared)
    cc1_in = [nc.dram_tensor(f"cc1_in{i}", [P, N1 // P // 2], F32) for i in (0, 1)]
    cc1_out = [nc.dram_tensor(f"cc1_out{i}", [P, N1 // P // 2], F32,
                              addr_space="Shared") for i in (0, 1)]
    cc2_in = [nc.dram_tensor(f"cc2_in{i}", [P, N2 // P // 2], F32) for i in (0, 1)]
    cc2_out = [nc.dram_tensor(f"cc2_out{i}", [P, N2 // P // 2], F32,
                              addr_space="Shared") for i in (0, 1)]
    cc3_in = [nc.dram_tensor(f"cc3_in{i}", [P, N3 // P], F32) for i in (0, 1)]
    cc3_out = [nc.dram_tensor(f"cc3_out{i}", [P, N3 // P], F32,
                              addr_space="Shared") for i in (0, 1)]

    rg = [list(range(NCORES))]
    ccw_in = nc.dram_tensor("ccw_in", [P, 32], F32)
    ccw_out = nc.dram_tensor("ccw_out", [P, 32], F32, addr_space="Shared")

    with TileContext(nc) as tc:
        with (
            tc.tile_pool(name="big", bufs=1) as big,      # xdig / h (8 MB slot)
            tc.tile_pool(name="act", bufs=2) as actp,     # binarized activations
            tc.tile_pool(name="wp", bufs=4) as wp,        # streamed weights
            tc.tile_pool(name="sm", bufs=2) as sm,        # small stuff
            tc.tile_pool(name="tmp", bufs=3) as tmpp,     # binarize temps
            tc.tile_pool(name="ps", bufs=2, space="PSUM") as ps,
        ):
            def act_tiles(gen):
                # 16 pair tiles [P, 2, B] fp8 per layer boundary; tags shared
                # across generations so a3 reuses a1's slots (bufs=2).
                return [actp.tile([P, 2, B], FP8, tag=f"act{i}", name=f"a{gen}_{i}")
                        for i in range(16)]

            def binarize(dst, src, mu_ap, nmu_ap, on_act):
                # dst (fp8) = sign(src - mu), with sign(0) = 0 exactly.
                # ACT Sign LUT is exact (probed: 0 -> 0, correct at 2.4e-4
                # margins and up to 2^31); split across ACT and DVE so both
                # engines binarize in parallel at layer boundaries.
                if on_act:
                    nc.scalar.activation(dst, src,
                                         mybir.ActivationFunctionType.Sign,
                                         bias=nmu_ap if nmu_ap is not None else 0.0)
                    return
                t = tmpp.tile([P, B], mybir.dt.bfloat16, tag="bin", name="bin")
                if mu_ap is None:
                    nc.vector.tensor_scalar(t[:], src, BIG, None,
                                            mybir.AluOpType.mult)
                else:
                    nc.vector.tensor_scalar(t[:], src, mu_ap, BIG,
                                            mybir.AluOpType.subtract,
                                            mybir.AluOpType.mult)
                nc.vector.tensor_scalar(dst, t[:], -1.0, 1.0,
                                        mybir.AluOpType.max, mybir.AluOpType.min)

            # ncfw warmup collective (overlaps with layer 0 compute)
            nc.gpsimd.collective_compute(
                "AllReduce", mybir.AluOpType.add, replica_groups=rg,
                ins=[ccw_in[:]], outs=[ccw_out[:]],
            )

            # ---------------- layer 0 ----------------
            xdig = big.tile([P, 3, K0 // P, B], F16, tag="big")  # 6 MB
            for c in range(K0 // P):
                for j in range(3):
                    nc.sync.dma_start(out=xdig[:, j, c, :], in_=xd[j][:, c, :])

            a1 = act_tiles(1)
            for nt in range(N0 // P):
                wt = wp.tile([P, K0 // P, P], F16, tag="w0", name="wt0")
                nc.scalar.dma_start(out=wt[:], in_=w0[nt])
                pj = [ps.tile([P, B], F32, tag=f"ps{j}", name=f"pj{j}")
                      for j in range(3)]
                for c in range(K0 // P):
                    for j in range(3):
                        nc.tensor.matmul(
                            pj[j][:], wt[:, c, :], xdig[:, j, c, :],
                            start=(c == 0), stop=(c == K0 // P - 1),
                        )
                acc = tmpp.tile([P, B], F32, tag="acc", name="acc")
                nc.vector.tensor_scalar(acc[:], pj[2][:], BASE, None,
                                        mybir.AluOpType.mult)
                nc.vector.tensor_tensor(acc[:], acc[:], pj[1][:],
                                        mybir.AluOpType.add)
                nc.vector.tensor_scalar(acc[:], acc[:], BASE, None,
                                        mybir.AluOpType.mult)
                nc.vector.tensor_tensor(acc[:], acc[:], pj[0][:],
                                        mybir.AluOpType.add)
                binarize(a1[nt // 2][:, nt % 2, :], acc[:], None,
                         None, nt % 2 == 0)

            # ---------------- layers 1 and 2 (fp8 DoubleRow) --------------
            def mid_layer(a_in, w, K, N, cc_in, cc_out, gen):
                nk2 = K // P // 2
                nn = N // P
                h = big.tile([P, nn, B], F32, tag="big", name=f"h{gen}")
                stats = sm.tile([P, nn], F32, tag="stats", name=f"st{gen}")
                for nt in range(nn):
                    wt = wp.tile([P, K // P, P], FP8, tag="w", name=f"wt{gen}")
                    nc.scalar.dma_start(out=wt[:], in_=w[nt])
                    acc = ps.tile([P, B], F32, tag="psA", name=f"ac{gen}")
                    for c in range(nk2):
                        nc.tensor.matmul(acc[:], wt[:, 2 * c:2 * c + 2, :],
                                         a_in[c][:], perf_mode=DR,
                                         start=(c == 0), stop=(c == nk2 - 1))
                    nc.scalar.copy(out=h[:, nt, :], in_=acc[:])
                    nc.vector.tensor_reduce(stats[:, nt:nt + 1], acc[:],
                                            axis=mybir.AxisListType.X,
                                            op=mybir.AluOpType.add)
                    if nt == nn // 2 - 1:
                        nc.sync.dma_start(out=cc_in[0][:],
                                          in_=stats[:, :nn // 2])
                        nc.gpsimd.collective_compute(
                            "AllReduce", mybir.AluOpType.add,
                            replica_groups=rg,
                            ins=[cc_in[0][:]], outs=[cc_out[0][:]],
                        )
                nc.sync.dma_start(out=cc_in[1][:], in_=stats[:, nn // 2:])
                nc.gpsimd.collective_compute(
                    "AllReduce", mybir.AluOpType.add, replica_groups=rg,
                    ins=[cc_in[1][:]], outs=[cc_out[1][:]],
                )
                gs = sm.tile([P, nn], F32, tag="gstats", name=f"gs{gen}")
                nc.sync.dma_start(out=gs[:, :nn // 2], in_=cc_out[0][:])
                nc.sync.dma_start(out=gs[:, nn // 2:], in_=cc_out[1][:])
                mu = sm.tile([P, nn], F32, tag="mu", name=f"mu{gen}")
                nc.vector.tensor_scalar(mu[:], gs[:], 1.0 / BATCH, None,
                                        mybir.AluOpType.mult)
                nmu = sm.tile([P, nn], F32, tag="nmu", name=f"nmu{gen}")
                nc.vector.tensor_scalar(nmu[:], gs[:], -1.0 / BATCH, None,
                                        mybir.AluOpType.mult)
                a_out = act_tiles(gen + 1)
                for nt in range(nn):
                    binarize(a_out[nt // 2][:, nt % 2, :], h[:, nt, :],
                             mu[:, nt:nt + 1], nmu[:, nt:nt + 1], nt % 2 == 0)
                return a_out

            a2 = mid_layer(a1, w1, K1, N1, cc1_in, cc1_out, 1)
            a3 = mid_layer(a2, w2, K2, N2, cc2_in, cc2_out, 2)

            # ---------------- layer 3 (fp8 DoubleRow) ----------------
            nk2 = K3 // P // 2
            nn = N3 // P
            h = big.tile([P, nn, B], F32, tag="big", name="h3")
            st3 = sm.tile([P, 2 * nn], F32, tag="st3")
            for nt in range(nn):
                wt = wp.tile([P, K3 // P, P], FP8, tag="w", name="wt3")
                nc.scalar.dma_start(out=wt[:], in_=w3[nt])
                acc = ps.tile([P, B], F32, tag="psA", name="ac3")
                for c in range(nk2):
                    nc.tensor.matmul(acc[:], wt[:, 2 * c:2 * c + 2, :],
                                     a3[c][:], perf_mode=DR,
                                     start=(c == 0), stop=(c == nk2 - 1))
                nc.scalar.copy(out=h[:, nt, :], in_=acc[:])
                nc.vector.tensor_reduce(st3[:, nt:nt + 1], acc[:],
                                        axis=mybir.AxisListType.X,
                                        op=mybir.AluOpType.add)
                sq = tmpp.tile([P, B], F32, tag="sq", name="sq")
                nc.scalar.activation(sq[:], acc[:],
                                     mybir.ActivationFunctionType.Square)
                nc.vector.tensor_reduce(st3[:, nn + nt:nn + nt + 1], sq[:],
                                        axis=mybir.AxisListType.X,
                                        op=mybir.AluOpType.add)
                if nt == nn // 2 - 1:
                    nc.sync.dma_start(out=cc3_in[0][:, :nn // 2],
                                      in_=st3[:, :nn // 2])
                    nc.sync.dma_start(out=cc3_in[0][:, nn // 2:],
                                      in_=st3[:, nn:nn + nn // 2])
                    nc.gpsimd.collective_compute(
                        "AllReduce", mybir.AluOpType.add, replica_groups=rg,
                        ins=[cc3_in[0][:]], outs=[cc3_out[0][:]],
                    )
            nc.sync.dma_start(out=cc3_in[1][:, :nn // 2],
                              in_=st3[:, nn // 2:nn])
            nc.sync.dma_start(out=cc3_in[1][:, nn // 2:],
                              in_=st3[:, nn + nn // 2:])
            nc.gpsimd.collective_compute(
                "AllReduce", mybir.AluOpType.add, replica_groups=rg,
                ins=[cc3_in[1][:]], outs=[cc3_out[1][:]],
            )
            # BN tail in two halves so the first half's outputs flow while
            # the second stats AllReduce is still in flight.
            mu3 = sm.tile([P, nn], F32, tag="mu3")
            vpe = sm.tile([P, nn], F32, tag="vpe")
            musq = sm.tile([P, nn], F32, tag="musq")
            r = sm.tile([P, nn], F32, tag="r")
            rinv = sm.tile([P, nn], F32, tag="rinv")
            r2 = sm.tile([P, nn], F32, tag="r2")
            g3 = sm.tile([P, 2 * nn], F32, tag="g3")
            hn = nn // 2
            for half in (0, 1):
                sl = slice(0, hn) if half == 0 else slice(hn, nn)
                nc.sync.dma_start(out=g3[:, half * hn:half * hn + hn],
                                  in_=cc3_out[half][:, :hn])
                nc.sync.dma_start(out=g3[:, nn + half * hn:nn + half * hn + hn],
                                  in_=cc3_out[half][:, hn:])
                nc.vector.tensor_scalar(mu3[:, sl], g3[:, sl], 1.0 / BATCH,
                                        None, mybir.AluOpType.mult)
                qsl = slice(nn + half * hn, nn + half * hn + hn)
                nc.vector.tensor_scalar(vpe[:, sl], g3[:, qsl], 1.0 / BATCH,
                                        None, mybir.AluOpType.mult)
                nc.vector.tensor_tensor(musq[:, sl], mu3[:, sl], mu3[:, sl],
                                        mybir.AluOpType.mult)
                nc.vector.tensor_tensor(vpe[:, sl], vpe[:, sl], musq[:, sl],
                                        mybir.AluOpType.subtract)
                nc.vector.tensor_scalar(vpe[:, sl], vpe[:, sl], EPS, None,
                                        mybir.AluOpType.add)
                # r = sqrt(1/vpe) then one Newton step:
                # r = r*(1.5 - 0.5*vpe*r^2)
                nc.vector.reciprocal(rinv[:, sl], vpe[:, sl])
                nc.scalar.activation(r[:, sl], rinv[:, sl],
                                     mybir.ActivationFunctionType.Sqrt)
                nc.vector.tensor_tensor(r2[:, sl], r[:, sl], r[:, sl],
                                        mybir.AluOpType.mult)
                nc.vector.tensor_tensor(r2[:, sl], r2[:, sl], vpe[:, sl],
                                        mybir.AluOpType.mult)
                nc.vector.tensor_scalar(r2[:, sl], r2[:, sl], -0.5, 1.5,
                                        mybir.AluOpType.mult,
                                        mybir.AluOpType.add)
                nc.vector.tensor_tensor(r[:, sl], r[:, sl], r2[:, sl],
                                        mybir.AluOpType.mult)
                for nt in range(half * hn, half * hn + hn):
                    o = tmpp.tile([P, B], F32, tag="o", name="o")
                    nc.vector.tensor_scalar(o[:], h[:, nt, :],
                                            mu3[:, nt:nt + 1],
                                            r[:, nt:nt + 1],
                                            mybir.AluOpType.subtract,
                                            mybir.AluOpType.mult)
                    nc.sync.dma_start(out=out[nt], in_=o[:])
    nc.compile()
    return nc


def _prep_weights(W, n_pad, dtype):
    """sign(W) [N,K] -> [N/P, P, K/P, P] tiles of sign(W).T, cast to dtype."""
    N, K = W.shape
    S = np.sign(W).astype(np.float32)
    if n_pad != N:
        S = np.concatenate([S, np.zeros((n_pad - N, K), np.float32)], axis=0)
    # element [nt, p, c, m] = S.T[c*P+p, nt*P+m] = S[nt*P+m, c*P+p]
    A = S.reshape(n_pad // P, P, K // P, P)            # [nt, m, c, p]
    A = np.ascontiguousarray(A.transpose(0, 3, 2, 1))  # [nt, p, c, m]
    return A.astype(dtype)


def _host_reference(inputs):
    """Exact-semantics fallback (general g/b); matches jnp reference to ~1e-7."""
    x = inputs["x"].astype(np.float64)
    h = x
    for i in range(4):
        S = np.sign(inputs[f"W{i}"]).astype(np.float64)
        h = h @ S.T
        mu = h.mean(0)
        var = h.var(0)
        h = inputs[f"g{i}"] * (h - mu) / np.sqrt(var + EPS) + inputs[f"b{i}"]
        if i < 3:
            h = np.sign(h)
    return h.astype(np.float32)


def _prepare(inputs):
    x = inputs["x"]
    xm = x.astype(np.float64)
    xc = xm - xm.mean(0)
    FIX = np.rint(xc * SCALE).astype(np.int64)        # [BATCH, K0]
    d0 = ((FIX + 256) % 512) - 256
    cur = (FIX - d0) // 512
    d1 = ((cur + 256) % 512) - 256
    d2 = (cur - d1) // 512
    if np.abs(d2).max() > 2047:                        # fp16 exact-int bound
        return None
    planes = []
    for dj in (d0, d1, d2):
        # [BATCH, K0] -> per-core [P, K0/P, B] fp16, k-major
        t = dj.astype(np.float16).T                    # [K0, BATCH]
        t = t.reshape(K0 // P, P, BATCH)               # [c, p, batch]
        planes.append(np.ascontiguousarray(t.transpose(1, 0, 2)))  # [p, c, b]
    f8 = ml_dtypes.float8_e4m3
    w0 = _prep_weights(inputs["W0"], N0, np.float16)
    w1 = _prep_weights(inputs["W1"], N1, f8)
    w2 = _prep_weights(inputs["W2"], N2, f8)
    w3 = _prep_weights(inputs["W3"], N3, f8)
    in_maps = []
    for c in range(NCORES):
        sl = slice(c * B, (c + 1) * B)
        in_maps.append({
            "xd0": np.ascontiguousarray(planes[0][:, :, sl]),
            "xd1": np.ascontiguousarray(planes[1][:, :, sl]),
            "xd2": np.ascontiguousarray(planes[2][:, :, sl]),
            "w0": w0, "w1": w1, "w2": w2, "w3": w3,
        })
    return in_maps


def _assemble(results):
    outs = []
    for c in range(NCORES):
        o = results[c]["out"].reshape(N3, B)           # [1024, 512]
        outs.append(o[:1000].T)                        # [512, 1000]
    return np.ascontiguousarray(np.concatenate(outs, axis=0).astype(np.float32))


def _shapes_ok(inputs):
    try:
        if inputs["x"].shape != (BATCH, K0):
            return False
        for i, (n, k) in enumerate(((N0, K0), (N1, K1), (N2, K2), (1000, K3))):
            if inputs[f"W{i}"].shape != (n, k):
                return False
    except Exception:
        return False
    return True


def run(inputs, trace=False):
    inputs = {k: np.asarray(v) for k, v in inputs.items()}
    generic = not _shapes_ok(inputs)
    for i in range(4):
        if not (np.all(inputs[f"g{i}"] == 1.0) and np.all(inputs[f"b{i}"] == 0.0)):
            generic = True
    if generic:
        return _host_reference(inputs), None
    in_maps = _prepare(inputs)
    if in_maps is None:
        return _host_reference(inputs), None
    try:
        if "nc" not in _CACHE:
            _CACHE["nc"] = _build_nc()
        res = run_bass_kernel_spmd(_CACHE["nc"], in_maps,
                                   core_ids=list(range(NCORES)), trace=trace)
        return _assemble(res.results), res
    except Exception as e:  # device/toolchain unavailable: host fallback
        import sys
        print(f"kernel: device path failed ({type(e).__name__}: {e}); "
              "falling back to host reference", file=sys.stderr)
        return _host_reference(inputs), None


def kernel(**inputs):
    out, _ = run(inputs)
    return out


if __name__ == "__main__":
    d = np.load("/root/problem/inputs.npz")
    inputs = {k: d[k] for k in d.files}
    out, res = run(inputs, trace=False)
    ref = np.load("/root/problem/ref_neuron.npy")
    rel = np.linalg.norm((out - ref).astype(np.float64)) / np.linalg.norm(
        ref.astype(np.float64))
    print("Relative error:", rel)


# revision 15
# speedup vs baseline: 1.1248x; 1.1248x over previous
"""Trainium2 Bass kernel for the binarized-MLP (BNN) forward pass.

Strategy (data-parallel over batch, 8 cores x 512 rows):

  Layer 0 (fp32 x @ sign(W0).T, then batchnorm + sign):
    By linearity, h0 - mean_batch(h0) == (x - colmean(x)) @ sign(W0).T, so the
    batch mean never needs to be computed on device for layer 0 (no
    collective).  The centered x is quantized host-side to fixed point
    (scale 2^24) and split into three base-512 signed digit planes, each
    exactly representable in fp16.  Three fp16 matmuls accumulate EXACT small
    integers in fp32 PSUM (|plane sum| < 2^19 << 2^24, so no rounding ever),
    and a Horner recombine (acc = (P2*512 + P1)*512 + P0) on the vector
    engine reproduces the exact sign of h0 - mu0 for every element
    (host-verified: 0 flips, min margin 44 fixed-point units).
  Layers 1..3: activations and sign(W) are {-1,0,+1}; products and sums are
    small integers, so fp8(e4m3) DoubleRow matmuls with fp32 PSUM
    accumulation are EXACT.  Batch stats are integer column sums, allreduced
    in two waves per layer (the first wave's collective hides under the
    second half's matmuls); mu = sum/4096 is exact in fp32.  Binarize
    alternates between the ACT engine's Sign LUT (probed exact, incl.
    sign(0)=0) and a DVE clamp((h-mu)*2^50, -1, 1) pair so both engines
    binarize in parallel at layer boundaries.
  Layer 3 output: out = (h3 - mu3) * rsqrt(var3 + eps) with var from the
    allreduced sum/sumsq, one Newton step after reciprocal+sqrt, computed in
    two halves so the first half's outputs stream during the second
    collective.  Measured: ~0.70-0.82 ms on 8 TRN2 NeuronCores,
    relative error 1.2e-7 vs the jax reference.

The kernel() entry takes FULL unsharded inputs and returns the FULL output.
"""

import numpy as np
import ml_dtypes

import concourse.bass as bass
from concourse import bacc
import concourse.mybir as mybir
from concourse.tile import TileContext
from concourse.bass_utils import run_bass_kernel_spmd

P = 128
B = 512                # per-core batch (4096 / 8)
NCORES = 8
BATCH = 4096
EPS = 1e-5
SCALE = float(2 ** 24)  # fixed-point scale for centered x
BASE = 512.0            # digit base
BIG = float(2 ** 50)    # sign-extraction multiplier

K0, N0 = 2048, 4096     # layer 0
K1, N1 = 4096, 4096
K2, N2 = 4096, 4096
K3, N3 = 4096, 1024     # N3 padded 1000 -> 1024

F32 = mybir.dt.float32
F16 = mybir.dt.float16
FP8 = mybir.dt.float8e4
DR = mybir.MatmulPerfMode.DoubleRow

_CACHE = {}


def _build_nc():
    nc = bacc.Bacc(num_devices=NCORES)

    xd = [nc.declare_dram_parameter(f"xd{j}", [P, K0 // P, B], F16, isOutput=False)
          for j in range(3)]
    w0 = nc.declare_dram_parameter("w0", [N0 // P, P, K0 // P, P], F16, isOutput=False)
    w1 = nc.declare_dram_parameter("w1", [N1 // P, P, K1 // P, P], FP8, isOutput=False)
    w2 = nc.declare_dram_parameter("w2", [N2 // P, P, K2 // P, P], FP8, isOutput=False)
    w3 = nc.declare_dram_parameter("w3", [N3 // P, P, K3 // P, P], FP8, isOutput=False)
    out = nc.declare_dram_parameter("out", [N3 // P, P, B], F32, isOutput=True)

    # collective bounce buffers (internal DRAM; output must be Shared)
    cc1_in = [nc.dram_tensor(f"cc1_in{i}", [P, N1 // P // 2], F32) for i in (0, 1)]
    cc1_out = [nc.dram_tensor(f"cc1_out{i}", [P, N1 // P // 2], F32,
                              addr_space="Shared") for i in (0, 1)]
    cc2_in = [nc.dram_tensor(f"cc2_in{i}", [P, N2 // P // 2], F32) for i in (0, 1)]
    cc2_out = [nc.dram_tensor(f"cc2_out{i}", [P, N2 // P // 2], F32,
                              addr_space="Shared") for i in (0, 1)]
    cc3_in = [nc.dram_tensor(f"cc3_in{i}", [P, N3 // P], F32) for i in (0, 1)]
    cc3_out = [nc.dram_tensor(f"cc3_out{i}", [P, N3 // P], F32,
                              addr_space="Shared") for i in (0, 1)]

    rg = [list(range(NCORES))]
    ccw_in = nc.dram_tensor("ccw_in", [P, 32], F32)
    ccw_out = nc.dram_tensor("ccw_out", [P, 32], F32, addr_space="Shared")

    with TileContext(nc) as tc:
        with (
            tc.tile_pool(name="big", bufs=1) as big,      # xdig / h (8 MB slot)
            tc.tile_pool(name="act", bufs=2) as actp,     # binarized activations
            tc.tile_pool(name="wp", bufs=4) as wp,        # streamed weights
            tc.tile_pool(name="sm", bufs=2) as sm,        # small stuff
            tc.tile_pool(name="tmp", bufs=3) as tmpp,     # binarize temps
            tc.tile_pool(name="ps", bufs=2, space="PSUM") as ps,
        ):
            def act_tiles(gen):
                # 16 pair tiles [P, 2, B] fp8 per layer boundary; tags shared
                # across generations so a3 reuses a1's slots (bufs=2).
                return [actp.tile([P, 2, B], FP8, tag=f"act{i}", name=f"a{gen}_{i}")
                        for i in range(16)]

            def binarize(dst, src, mu_ap, nmu_ap, on_act):
                # dst (fp8) = sign(src - mu), with sign(0) = 0 exactly.
                # ACT Sign LUT is exact (probed: 0 -> 0, correct at 2.4e-4
                # margins and up to 2^31); split across ACT and DVE so both
                # engines binarize in parallel at layer boundaries.
                if on_act:
                    nc.scalar.activation(dst, src,
                                         mybir.ActivationFunctionType.Sign,
                                         bias=nmu_ap if nmu_ap is not None else 0.0)
                    return
                t = tmpp.tile([P, B], mybir.dt.bfloat16, tag="bin", name="bin")
                if mu_ap is None:
                    nc.vector.tensor_scalar(t[:], src, BIG, None,
                                            mybir.AluOpType.mult)
                else:
                    nc.vector.tensor_scalar(t[:], src, mu_ap, BIG,
                                            mybir.AluOpType.subtract,
                                            mybir.AluOpType.mult)
                nc.vector.tensor_scalar(dst, t[:], -1.0, 1.0,
                                        mybir.AluOpType.max, mybir.AluOpType.min)

            # ncfw warmup collective (overlaps with layer 0 compute)
            nc.gpsimd.collective_compute(
                "AllReduce", mybir.AluOpType.add, replica_groups=rg,
                ins=[ccw_in[:]], outs=[ccw_out[:]],
            )

            # ---------------- layer 0 ----------------
            xdig = big.tile([P, 3, K0 // P, B], F16, tag="big")  # 6 MB
            for c in range(K0 // P):
                for j in range(3):
                    nc.sync.dma_start(out=xdig[:, j, c, :], in_=xd[j][:, c, :])

            a1 = act_tiles(1)
            for nt in range(N0 // P):
                wt = wp.tile([P, K0 // P, P], F16, tag="w0", name="wt0")
                nc.scalar.dma_start(out=wt[:], in_=w0[nt])
                pj = [ps.tile([P, B], F32, tag=f"ps{j}", name=f"pj{j}")
                      for j in range(3)]
                for c in range(K0 // P):
                    for j in range(3):
                        nc.tensor.matmul(
                            pj[j][:], wt[:, c, :], xdig[:, j, c, :],
                            start=(c == 0), stop=(c == K0 // P - 1),
                        )
                acc = tmpp.tile([P, B], F32, tag="acc", name="acc")
                nc.vector.tensor_scalar(acc[:], pj[2][:], BASE, None,
                                        mybir.AluOpType.mult)
                nc.vector.tensor_tensor(acc[:], acc[:], pj[1][:],
                                        mybir.AluOpType.add)
                nc.vector.tensor_scalar(acc[:], acc[:], BASE, None,
                                        mybir.AluOpType.mult)
                nc.vector.tensor_tensor(acc[:], acc[:], pj[0][:],
                                        mybir.AluOpType.add)
                binarize(a1[nt // 2][:, nt % 2, :], acc[:], None,
                         None, nt % 2 == 0)

            # ---------------- layers 1 and 2 (fp8 DoubleRow) --------------
            def mid_layer(a_in, w, K, N, cc_in, cc_out, gen):
                nk2 = K // P // 2
                nn = N // P
                h = big.tile([P, nn, B], F32, tag="big", name=f"h{gen}")
                stats = sm.tile([P, nn], F32, tag="stats", name=f"st{gen}")
                for nt in range(nn):
                    wt = wp.tile([P, K // P, P], FP8, tag="w", name=f"wt{gen}")
                    nc.scalar.dma_start(out=wt[:], in_=w[nt])
                    acc = ps.tile([P, B], F32, tag="psA", name=f"ac{gen}")
                    for c in range(nk2):
                        nc.tensor.matmul(acc[:], wt[:, 2 * c:2 * c + 2, :],
                                         a_in[c][:], perf_mode=DR,
                                         start=(c == 0), stop=(c == nk2 - 1))
                    nc.scalar.copy(out=h[:, nt, :], in_=acc[:])
                    nc.vector.tensor_reduce(stats[:, nt:nt + 1], acc[:],
                                            axis=mybir.AxisListType.X,
                                            op=mybir.AluOpType.add)
                    if nt == nn // 2 - 1:
                        nc.sync.dma_start(out=cc_in[0][:],
                                          in_=stats[:, :nn // 2])
                        nc.gpsimd.collective_compute(
                            "AllReduce", mybir.AluOpType.add,
                            replica_groups=rg,
                            ins=[cc_in[0][:]], outs=[cc_out[0][:]],
                        )
                nc.sync.dma_start(out=cc_in[1][:], in_=stats[:, nn // 2:])
                nc.gpsimd.collective_compute(
                    "AllReduce", mybir.AluOpType.add, replica_groups=rg,
                    ins=[cc_in[1][:]], outs=[cc_out[1][:]],
                )
                gs = sm.tile([P, nn], F32, tag="gstats", name=f"gs{gen}")
                nc.sync.dma_start(out=gs[:, :nn // 2], in_=cc_out[0][:])
                nc.sync.dma_start(out=gs[:, nn // 2:], in_=cc_out[1][:])
                mu = sm.tile([P, nn], F32, tag="mu", name=f"mu{gen}")
                nc.vector.tensor_scalar(mu[:], gs[:], 1.0 / BATCH, None,
                                        mybir.AluOpType.mult)
                nmu = sm.tile([P, nn], F32, tag="nmu", name=f"nmu{gen}")
                nc.vector.tensor_scalar(nmu[:], gs[:], -1.0 / BATCH, None,
                                        mybir.AluOpType.mult)
                a_out = act_tiles(gen + 1)
                for nt in range(nn):
                    binarize(a_out[nt // 2][:, nt % 2, :], h[:, nt, :],
                             mu[:, nt:nt + 1], nmu[:, nt:nt + 1], nt % 2 == 0)
                return a_out

            a2 = mid_layer(a1, w1, K1, N1, cc1_in, cc1_out, 1)
            a3 = mid_layer(a2, w2, K2, N2, cc2_in, cc2_out, 2)

            # ---------------- layer 3 (fp8 DoubleRow) ----------------
            nk2 = K3 // P // 2
            nn = N3 // P
            h = big.tile([P, nn, B], F32, tag="big", name="h3")
            st3 = sm.tile([P, 2 * nn], F32, tag="st3")
            for nt in range(nn):
                wt = wp.tile([P, K3 // P, P], FP8, tag="w", name="wt3")
                nc.scalar.dma_start(out=wt[:], in_=w3[nt])
                acc = ps.tile([P, B], F32, tag="psA", name="ac3")
                for c in range(nk2):
                    nc.tensor.matmul(acc[:], wt[:, 2 * c:2 * c + 2, :],
                                     a3[c][:], perf_mode=DR,
                                     start=(c == 0), stop=(c == nk2 - 1))
                nc.scalar.copy(out=h[:, nt, :], in_=acc[:])
                nc.vector.tensor_reduce(st3[:, nt:nt + 1], acc[:],
                                        axis=mybir.AxisListType.X,
                                        op=mybir.AluOpType.add)
                sq = tmpp.tile([P, B], F32, tag="sq", name="sq")
                nc.scalar.activation(sq[:], acc[:],
                                     mybir.ActivationFunctionType.Square)
                nc.vector.tensor_reduce(st3[:, nn + nt:nn + nt + 1], sq[:],
                                        axis=mybir.AxisListType.X,
                                        op=mybir.AluOpType.add)
                if nt == nn // 2 - 1:
                    nc.sync.dma_start(out=cc3_in[0][:, :nn // 2],
                                      in_=st3[:, :nn // 2])
                    nc.sync.dma_start(out=cc3_in[0][:, nn // 2:],
                                      in_=st3[:, nn:nn + nn // 2])
                    nc.gpsimd.collective_compute(
                        "AllReduce", mybir.AluOpType.add, replica_groups=rg,
                        ins=[cc3_in[0][:]], outs=[cc3_out[0][:]],
                    )
            nc.sync.dma_start(out=cc3_in[1][:, :nn // 2],
                              in_=st3[:, nn // 2:nn])
            nc.sync.dma_start(out=cc3_in[1][:, nn // 2:],
                              in_=st3[:, nn + nn // 2:])
            nc.gpsimd.collective_compute(
                "AllReduce", mybir.AluOpType.add, replica_groups=rg,
                ins=[cc3_in[1][:]], outs=[cc3_out[1][:]],
            )
            # BN tail in two halves so the first half's outputs flow while
            # the second stats AllReduce is still in flight.
            mu3 = sm.tile([P, nn], F32, tag="mu3")
            vpe = sm.tile([P, nn], F32, tag="vpe")
            musq = sm.tile([P, nn], F32, tag="musq")
            r = sm.tile([P, nn], F32, tag="r")
            rinv = sm.tile([P, nn], F32, tag="rinv")
            r2 = sm.tile([P, nn], F32, tag="r2")
            g3 = sm.tile([P, 2 * nn], F32, tag="g3")
            hn = nn // 2
            for half in (0, 1):
                sl = slice(0, hn) if half == 0 else slice(hn, nn)
                nc.sync.dma_start(out=g3[:, half * hn:half * hn + hn],
                                  in_=cc3_out[half][:, :hn])
                nc.sync.dma_start(out=g3[:, nn + half * hn:nn + half * hn + hn],
                                  in_=cc3_out[half][:, hn:])
                nc.vector.tensor_scalar(mu3[:, sl], g3[:, sl], 1.0 / BATCH,
                                        None, mybir.AluOpType.mult)
                qsl = slice(nn + half * hn, nn + half * hn + hn)
                nc.vector.tensor_scalar(vpe[:, sl], g3[:, qsl], 1.0 / BATCH,
                                        None, mybir.AluOpType.mult)
                nc.vector.tensor_tensor(musq[:, sl], mu3[:, sl], mu3[:, sl],
                                        mybir.AluOpType.mult)
                nc.vector.tensor_tensor(vpe[:, sl], vpe[:, sl], musq[:, sl],
                                        mybir.AluOpType.subtract)
                nc.vector.tensor_scalar(vpe[:, sl], vpe[:, sl], EPS, None,
                                        mybir.AluOpType.add)
                # r = sqrt(1/vpe) then one Newton step:
                # r = r*(1.5 - 0.5*vpe*r^2)
                nc.vector.reciprocal(rinv[:, sl], vpe[:, sl])
                nc.scalar.activation(r[:, sl], rinv[:, sl],
                                     mybir.ActivationFunctionType.Sqrt)
                nc.vector.tensor_tensor(r2[:, sl], r[:, sl], r[:, sl],
                                        mybir.AluOpType.mult)
                nc.vector.tensor_tensor(r2[:, sl], r2[:, sl], vpe[:, sl],
                                        mybir.AluOpType.mult)
                nc.vector.tensor_scalar(r2[:, sl], r2[:, sl], -0.5, 1.5,
                                        mybir.AluOpType.mult,
                                        mybir.AluOpType.add)
                nc.vector.tensor_tensor(r[:, sl], r[:, sl], r2[:, sl],
                                        mybir.AluOpType.mult)
                for nt in range(half * hn, half * hn + hn):
                    o = tmpp.tile([P, B], F32, tag="o", name="o")
                    nc.vector.tensor_scalar(o[:], h[:, nt, :],
                                            mu3[:, nt:nt + 1],
                                            r[:, nt:nt + 1],
                                            mybir.AluOpType.subtract,
                                            mybir.AluOpType.mult)
                    nc.sync.dma_start(out=out[nt], in_=o[:])
    nc.compile()
    return nc


def _prep_weights(W, n_pad, dtype):
    """sign(W) [N,K] -> [N/P, P, K/P, P] tiles of sign(W).T, cast to dtype."""
    N, K = W.shape
    S = np.sign(W).astype(np.float32)
    if n_pad != N:
        S = np.concatenate([S, np.zeros((n_pad - N, K), np.float32)], axis=0)
    # element [nt, p, c, m] = S.T[c*P+p, nt*P+m] = S[nt*P+m, c*P+p]
    A = S.reshape(n_pad // P, P, K // P, P)            # [nt, m, c, p]
    A = np.ascontiguousarray(A.transpose(0, 3, 2, 1))  # [nt, p, c, m]
    return A.astype(dtype)


def _host_reference(inputs):
    """Exact-semantics fallback (general g/b); matches jnp reference to ~1e-7."""
    x = inputs["x"].astype(np.float64)
    h = x
    for i in range(4):
        S = np.sign(inputs[f"W{i}"]).astype(np.float64)
        h = h @ S.T
        mu = h.mean(0)
        var = h.var(0)
        h = inputs[f"g{i}"] * (h - mu) / np.sqrt(var + EPS) + inputs[f"b{i}"]
        if i < 3:
            h = np.sign(h)
    return h.astype(np.float32)


def _prepare(inputs):
    x = inputs["x"]
    xm = x.astype(np.float64)
    xc = xm - xm.mean(0)
    FIX = np.rint(xc * SCALE).astype(np.int64)        # [BATCH, K0]
    d0 = ((FIX + 256) % 512) - 256
    cur = (FIX - d0) // 512
    d1 = ((cur + 256) % 512) - 256
    d2 = (cur - d1) // 512
    if np.abs(d2).max() > 2047:                        # fp16 exact-int bound
        return None
    planes = []
    for dj in (d0, d1, d2):
        # [BATCH, K0] -> per-core [P, K0/P, B] fp16, k-major
        t = dj.astype(np.float16).T                    # [K0, BATCH]
        t = t.reshape(K0 // P, P, BATCH)               # [c, p, batch]
        planes.append(np.ascontiguousarray(t.transpose(1, 0, 2)))  # [p, c, b]
    f8 = ml_dtypes.float8_e4m3
    w0 = _prep_weights(inputs["W0"], N0, np.float16)
    w1 = _prep_weights(inputs["W1"], N1, f8)
    w2 = _prep_weights(inputs["W2"], N2, f8)
    w3 = _prep_weights(inputs["W3"], N3, f8)
    in_maps = []
    for c in range(NCORES):
        sl = slice(c * B, (c + 1) * B)
        in_maps.append({
            "xd0": np.ascontiguousarray(planes[0][:, :, sl]),
            "xd1": np.ascontiguousarray(planes[1][:, :, sl]),
            "xd2": np.ascontiguousarray(planes[2][:, :, sl]),
            "w0": w0, "w1": w1, "w2": w2, "w3": w3,
        })
    return in_maps


def _assemble(results):
    outs = []
    for c in range(NCORES):
        o = results[c]["out"].reshape(N3, B)           # [1024, 512]
        outs.append(o[:1000].T)                        # [512, 1000]
    return np.ascontiguousarray(np.concatenate(outs, axis=0).astype(np.float32))


def _shapes_ok(inputs):
    try:
        if inputs["x"].shape != (BATCH, K0):
            return False
        for i, (n, k) in enumerate(((N0, K0), (N1, K1), (N2, K2), (1000, K3))):
            if inputs[f"W{i}"].shape != (n, k):
                return False
    except Exception:
        return False
    return True


def run(inputs, trace=False):
    inputs = {k: np.asarray(v) for k, v in inputs.items()}
    generic = not _shapes_ok(inputs)
    for i in range(4):
        if not (np.all(inputs[f"g{i}"] == 1.0) and np.all(inputs[f"b{i}"] == 0.0)):
            generic = True
    if generic:
        return _host_reference(inputs), None
    in_maps = _prepare(inputs)
    if in_maps is None:
        return _host_reference(inputs), None
    try:
        if "nc" not in _CACHE:
            _CACHE["nc"] = _build_nc()
        res = run_bass_kernel_spmd(_CACHE["nc"], in_maps,
                                   core_ids=list(range(NCORES)), trace=trace)
        return _assemble(res.results), res
    except Exception as e:  # device/toolchain unavailable: host fallback
        import sys
        print(f"kernel: device path failed ({type(e).__name__}: {e}); "
              "falling back to host reference", file=sys.stderr)
        return _host_reference(inputs), None


def kernel(**inputs):
    out, _ = run(inputs)
    return out


if __name__ == "__main__":
    d = np.load("/root/problem/inputs.npz")
    inputs = {k: d[k] for k in d.files}
    out, res = run(inputs, trace=False)
    ref = np.load("/root/problem/ref_neuron.npy")
    rel = np.linalg.norm((out - ref).astype(np.float64)) / np.linalg.norm(
        ref.astype(np.float64))
    print("Relative error:", rel)


# revision 17
# speedup vs baseline: 1.1320x; 1.0064x over previous
"""Trainium2 Bass kernel for the binarized-MLP (BNN) forward pass.

Strategy (data-parallel over batch, 8 cores x 512 rows):

  Layer 0 (fp32 x @ sign(W0).T, then batchnorm + sign):
    By linearity, h0 - mean_batch(h0) == (x - colmean(x)) @ sign(W0).T, so the
    batch mean never needs to be computed on device for layer 0 (no
    collective).  The centered x is quantized host-side to fixed point
    (scale 2^24) and split into three base-512 signed digit planes, each
    exactly representable in fp16.  Three fp16 matmuls accumulate EXACT small
    integers in fp32 PSUM (|plane sum| < 2^19 << 2^24, so no rounding ever),
    and a Horner recombine (acc = (P2*512 + P1)*512 + P0) on the vector
    engine reproduces the exact sign of h0 - mu0 for every element
    (host-verified: 0 flips, min margin 44 fixed-point units).
  Layers 1..3: activations and sign(W) are {-1,0,+1}; products and sums are
    small integers, so fp8(e4m3) DoubleRow matmuls with fp32 PSUM
    accumulation are EXACT.  Batch stats are integer column sums, allreduced
    in two waves per layer (the first wave's collective hides under the
    second half's matmuls); mu = sum/4096 is exact in fp32.  Binarize
    alternates between the ACT engine's Sign LUT (probed exact, incl.
    sign(0)=0) and a DVE clamp((h-mu)*2^50, -1, 1) pair so both engines
    binarize in parallel at layer boundaries.
  Layer 3 output: out = (h3 - mu3) * rsqrt(var3 + eps) with var from the
    allreduced sum/sumsq, one Newton step after reciprocal+sqrt, computed in
    two halves so the first half's outputs stream during the second
    collective.  Measured: ~0.70-0.82 ms on 8 TRN2 NeuronCores,
    relative error 1.2e-7 vs the jax reference.

The kernel() entry takes FULL unsharded inputs and returns the FULL output.
"""

import numpy as np
import ml_dtypes

import concourse.bass as bass
from concourse import bacc
import concourse.mybir as mybir
from concourse.tile import TileContext
from concourse.bass_utils import run_bass_kernel_spmd

P = 128
B = 512                # per-core batch (4096 / 8)
NCORES = 8
BATCH = 4096
EPS = 1e-5
SCALE = float(2 ** 24)  # fixed-point scale for centered x
BASE = 512.0            # digit base
BIG = float(2 ** 50)    # sign-extraction multiplier

K0, N0 = 2048, 4096     # layer 0
K1, N1 = 4096, 4096
K2, N2 = 4096, 4096
K3, N3 = 4096, 1024     # N3 padded 1000 -> 1024

F32 = mybir.dt.float32
F16 = mybir.dt.float16
FP8 = mybir.dt.float8e4
DR = mybir.MatmulPerfMode.DoubleRow

_CACHE = {}


def _build_nc():
    nc = bacc.Bacc(num_devices=NCORES)

    xd = [nc.declare_dram_parameter(f"xd{j}", [P, K0 // P, B], F16, isOutput=False)
          for j in range(3)]
    w0 = nc.declare_dram_parameter("w0", [N0 // P, P, K0 // P, P], F16, isOutput=False)
    w1 = nc.declare_dram_parameter("w1", [N1 // P, P, K1 // P, P], FP8, isOutput=False)
    w2 = nc.declare_dram_parameter("w2", [N2 // P, P, K2 // P, P], FP8, isOutput=False)
    w3 = nc.declare_dram_parameter("w3", [N3 // P, P, K3 // P, P], FP8, isOutput=False)
    out = nc.declare_dram_parameter("out", [N3 // P, P, B], F32, isOutput=True)

    # collective bounce buffers (internal DRAM; output must be Shared)
    SPLIT = 24              # mid-layer stats wave split (of 32 tiles)
    SPLIT3 = 6              # layer-3 stats wave split (of 8 tiles)
    cc1_in = [nc.dram_tensor("cc1_in0", [P, SPLIT], F32),
              nc.dram_tensor("cc1_in1", [P, 32 - SPLIT], F32)]
    cc1_out = [nc.dram_tensor("cc1_out0", [P, SPLIT], F32, addr_space="Shared"),
               nc.dram_tensor("cc1_out1", [P, 32 - SPLIT], F32,
                              addr_space="Shared")]
    cc2_in = [nc.dram_tensor("cc2_in0", [P, SPLIT], F32),
              nc.dram_tensor("cc2_in1", [P, 32 - SPLIT], F32)]
    cc2_out = [nc.dram_tensor("cc2_out0", [P, SPLIT], F32, addr_space="Shared"),
               nc.dram_tensor("cc2_out1", [P, 32 - SPLIT], F32,
                              addr_space="Shared")]
    cc3_in = [nc.dram_tensor("cc3_in0", [P, 2 * SPLIT3], F32),
              nc.dram_tensor("cc3_in1", [P, 2 * (8 - SPLIT3)], F32)]
    cc3_out = [nc.dram_tensor("cc3_out0", [P, 2 * SPLIT3], F32,
                              addr_space="Shared"),
               nc.dram_tensor("cc3_out1", [P, 2 * (8 - SPLIT3)], F32,
                              addr_space="Shared")]

    rg = [list(range(NCORES))]
    ccw_in = nc.dram_tensor("ccw_in", [P, 32], F32)
    ccw_out = nc.dram_tensor("ccw_out", [P, 32], F32, addr_space="Shared")

    with TileContext(nc) as tc:
        with (
            tc.tile_pool(name="big", bufs=1) as big,      # xdig / h (8 MB slot)
            tc.tile_pool(name="act", bufs=2) as actp,     # binarized activations
            tc.tile_pool(name="wp", bufs=4) as wp,        # streamed weights
            tc.tile_pool(name="sm", bufs=2) as sm,        # small stuff
            tc.tile_pool(name="tmp", bufs=3) as tmpp,     # binarize temps
            tc.tile_pool(name="ps", bufs=2, space="PSUM") as ps,
        ):
            def act_tiles(gen):
                # 16 pair tiles [P, 2, B] fp8 per layer boundary; tags shared
                # across generations so a3 reuses a1's slots (bufs=2).
                return [actp.tile([P, 2, B], FP8, tag=f"act{i}", name=f"a{gen}_{i}")
                        for i in range(16)]

            def binarize(dst, src, mu_ap, nmu_ap, on_act):
                # dst (fp8) = sign(src - mu), with sign(0) = 0 exactly.
                # ACT Sign LUT is exact (probed: 0 -> 0, correct at 2.4e-4
                # margins and up to 2^31); split across ACT and DVE so both
                # engines binarize in parallel at layer boundaries.
                if on_act:
                    nc.scalar.activation(dst, src,
                                         mybir.ActivationFunctionType.Sign,
                                         bias=nmu_ap if nmu_ap is not None else 0.0)
                    return
                t = tmpp.tile([P, B], mybir.dt.bfloat16, tag="bin", name="bin")
                if mu_ap is None:
                    nc.vector.tensor_scalar(t[:], src, BIG, None,
                                            mybir.AluOpType.mult)
                else:
                    nc.vector.tensor_scalar(t[:], src, mu_ap, BIG,
                                            mybir.AluOpType.subtract,
                                            mybir.AluOpType.mult)
                nc.vector.tensor_scalar(dst, t[:], -1.0, 1.0,
                                        mybir.AluOpType.max, mybir.AluOpType.min)

            # ncfw warmup collective (overlaps with layer 0 compute)
            nc.gpsimd.collective_compute(
                "AllReduce", mybir.AluOpType.add, replica_groups=rg,
                ins=[ccw_in[:]], outs=[ccw_out[:]],
            )

            # ---------------- layer 0 ----------------
            xdig = big.tile([P, 3, K0 // P, B], F16, tag="big")  # 6 MB
            for c in range(K0 // P):
                for j in range(3):
                    nc.sync.dma_start(out=xdig[:, j, c, :], in_=xd[j][:, c, :])

            a1 = act_tiles(1)
            for nt in range(N0 // P):
                wt = wp.tile([P, K0 // P, P], F16, tag="w0", name="wt0")
                nc.scalar.dma_start(out=wt[:], in_=w0[nt])
                pj = [ps.tile([P, B], F32, tag=f"ps{j}", name=f"pj{j}")
                      for j in range(3)]
                for c in range(K0 // P):
                    for j in range(3):
                        nc.tensor.matmul(
                            pj[j][:], wt[:, c, :], xdig[:, j, c, :],
                            start=(c == 0), stop=(c == K0 // P - 1),
                        )
                acc = tmpp.tile([P, B], F32, tag="acc", name="acc")
                nc.vector.tensor_scalar(acc[:], pj[2][:], BASE, None,
                                        mybir.AluOpType.mult)
                nc.vector.tensor_tensor(acc[:], acc[:], pj[1][:],
                                        mybir.AluOpType.add)
                nc.vector.tensor_scalar(acc[:], acc[:], BASE, None,
                                        mybir.AluOpType.mult)
                nc.vector.tensor_tensor(acc[:], acc[:], pj[0][:],
                                        mybir.AluOpType.add)
                binarize(a1[nt // 2][:, nt % 2, :], acc[:], None,
                         None, nt % 2 == 0)

            # ---------------- layers 1 and 2 (fp8 DoubleRow) --------------
            def mid_layer(a_in, w, K, N, cc_in, cc_out, gen):
                nk2 = K // P // 2
                nn = N // P
                h = big.tile([P, nn, B], F32, tag="big", name=f"h{gen}")
                stats = sm.tile([P, nn], F32, tag="stats", name=f"st{gen}")
                for nt in range(nn):
                    wt = wp.tile([P, K // P, P], FP8, tag="w", name=f"wt{gen}")
                    nc.scalar.dma_start(out=wt[:], in_=w[nt])
                    acc = ps.tile([P, B], F32, tag="psA", name=f"ac{gen}")
                    for c in range(nk2):
                        nc.tensor.matmul(acc[:], wt[:, 2 * c:2 * c + 2, :],
                                         a_in[c][:], perf_mode=DR,
                                         start=(c == 0), stop=(c == nk2 - 1))
                    nc.scalar.copy(out=h[:, nt, :], in_=acc[:])
                    nc.vector.tensor_reduce(stats[:, nt:nt + 1], acc[:],
                                            axis=mybir.AxisListType.X,
                                            op=mybir.AluOpType.add)
                    if nt == SPLIT - 1:
                        nc.sync.dma_start(out=cc_in[0][:],
                                          in_=stats[:, :SPLIT])
                        nc.gpsimd.collective_compute(
                            "AllReduce", mybir.AluOpType.add,
                            replica_groups=rg,
                            ins=[cc_in[0][:]], outs=[cc_out[0][:]],
                        )
                nc.sync.dma_start(out=cc_in[1][:], in_=stats[:, SPLIT:])
                nc.gpsimd.collective_compute(
                    "AllReduce", mybir.AluOpType.add, replica_groups=rg,
                    ins=[cc_in[1][:]], outs=[cc_out[1][:]],
                )
                a_out = act_tiles(gen + 1)
                # fully separate per-wave stat tiles so wave-A binarize
                # depends only on the first collective
                for wv, (lo, hi) in enumerate(((0, SPLIT), (SPLIT, nn))):
                    wn = hi - lo
                    gs = sm.tile([P, wn], F32, tag=f"gstats{wv}",
                                 name=f"gs{gen}_{wv}")
                    nc.sync.dma_start(out=gs[:], in_=cc_out[wv][:])
                    mu = sm.tile([P, wn], F32, tag=f"mu{wv}",
                                 name=f"mu{gen}_{wv}")
                    nc.vector.tensor_scalar(mu[:], gs[:], 1.0 / BATCH, None,
                                            mybir.AluOpType.mult)
                    nmu = sm.tile([P, wn], F32, tag=f"nmu{wv}",
                                  name=f"nmu{gen}_{wv}")
                    nc.vector.tensor_scalar(nmu[:], gs[:], -1.0 / BATCH, None,
                                            mybir.AluOpType.mult)
                    for nt in range(lo, hi):
                        i = nt - lo
                        binarize(a_out[nt // 2][:, nt % 2, :], h[:, nt, :],
                                 mu[:, i:i + 1], nmu[:, i:i + 1], nt % 2 == 0)
                return a_out

            a2 = mid_layer(a1, w1, K1, N1, cc1_in, cc1_out, 1)
            a3 = mid_layer(a2, w2, K2, N2, cc2_in, cc2_out, 2)

            # ---------------- layer 3 (fp8 DoubleRow) ----------------
            nk2 = K3 // P // 2
            nn = N3 // P
            h = big.tile([P, nn, B], F32, tag="big", name="h3")
            st3 = sm.tile([P, 2 * nn], F32, tag="st3")
            for nt in range(nn):
                wt = wp.tile([P, K3 // P, P], FP8, tag="w", name="wt3")
                nc.scalar.dma_start(out=wt[:], in_=w3[nt])
                acc = ps.tile([P, B], F32, tag="psA", name="ac3")
                for c in range(nk2):
                    nc.tensor.matmul(acc[:], wt[:, 2 * c:2 * c + 2, :],
                                     a3[c][:], perf_mode=DR,
                                     start=(c == 0), stop=(c == nk2 - 1))
                nc.scalar.copy(out=h[:, nt, :], in_=acc[:])
                nc.vector.tensor_reduce(st3[:, nt:nt + 1], acc[:],
                                        axis=mybir.AxisListType.X,
                                        op=mybir.AluOpType.add)
                sq = tmpp.tile([P, B], F32, tag="sq", name="sq")
                nc.scalar.activation(sq[:], acc[:],
                                     mybir.ActivationFunctionType.Square)
                nc.vector.tensor_reduce(st3[:, nn + nt:nn + nt + 1], sq[:],
                                        axis=mybir.AxisListType.X,
                                        op=mybir.AluOpType.add)
                if nt == SPLIT3 - 1:
                    nc.sync.dma_start(out=cc3_in[0][:, :SPLIT3],
                                      in_=st3[:, :SPLIT3])
                    nc.sync.dma_start(out=cc3_in[0][:, SPLIT3:],
                                      in_=st3[:, nn:nn + SPLIT3])
                    nc.gpsimd.collective_compute(
                        "AllReduce", mybir.AluOpType.add, replica_groups=rg,
                        ins=[cc3_in[0][:]], outs=[cc3_out[0][:]],
                    )
            nc.sync.dma_start(out=cc3_in[1][:, :nn - SPLIT3],
                              in_=st3[:, SPLIT3:nn])
            nc.sync.dma_start(out=cc3_in[1][:, nn - SPLIT3:],
                              in_=st3[:, nn + SPLIT3:])
            nc.gpsimd.collective_compute(
                "AllReduce", mybir.AluOpType.add, replica_groups=rg,
                ins=[cc3_in[1][:]], outs=[cc3_out[1][:]],
            )
            # BN tail per wave with fully separate stat tiles, so wave-0
            # outputs stream while the wave-1 AllReduce is in flight.
            for wv, (lo, hi) in enumerate(((0, SPLIT3), (SPLIT3, nn))):
                wn = hi - lo
                g3 = sm.tile([P, 2 * wn], F32, tag=f"g3_{wv}", name=f"g3_{wv}")
                nc.sync.dma_start(out=g3[:], in_=cc3_out[wv][:])
                mu3 = sm.tile([P, wn], F32, tag=f"mu3_{wv}", name=f"mu3_{wv}")
                vpe = sm.tile([P, wn], F32, tag=f"vpe_{wv}", name=f"vpe_{wv}")
                musq = sm.tile([P, wn], F32, tag=f"musq_{wv}",
                               name=f"musq_{wv}")
                r = sm.tile([P, wn], F32, tag=f"r_{wv}", name=f"r_{wv}")
                rinv = sm.tile([P, wn], F32, tag=f"rinv_{wv}",
                               name=f"rinv_{wv}")
                r2 = sm.tile([P, wn], F32, tag=f"r2_{wv}", name=f"r2_{wv}")
                nc.vector.tensor_scalar(mu3[:], g3[:, :wn], 1.0 / BATCH,
                                        None, mybir.AluOpType.mult)
                nc.vector.tensor_scalar(vpe[:], g3[:, wn:], 1.0 / BATCH,
                                        None, mybir.AluOpType.mult)
                nc.vector.tensor_tensor(musq[:], mu3[:], mu3[:],
                                        mybir.AluOpType.mult)
                nc.vector.tensor_tensor(vpe[:], vpe[:], musq[:],
                                        mybir.AluOpType.subtract)
                nc.vector.tensor_scalar(vpe[:], vpe[:], EPS, None,
                                        mybir.AluOpType.add)
                # r = sqrt(1/vpe) then one Newton step:
                # r = r*(1.5 - 0.5*vpe*r^2)
                nc.vector.reciprocal(rinv[:], vpe[:])
                nc.scalar.activation(r[:], rinv[:],
                                     mybir.ActivationFunctionType.Sqrt)
                nc.vector.tensor_tensor(r2[:], r[:], r[:],
                                        mybir.AluOpType.mult)
                nc.vector.tensor_tensor(r2[:], r2[:], vpe[:],
                                        mybir.AluOpType.mult)
                nc.vector.tensor_scalar(r2[:], r2[:], -0.5, 1.5,
                                        mybir.AluOpType.mult,
                                        mybir.AluOpType.add)
                nc.vector.tensor_tensor(r[:], r[:], r2[:],
                                        mybir.AluOpType.mult)
                for nt in range(lo, hi):
                    i = nt - lo
                    o = tmpp.tile([P, B], F32, tag="o", name="o")
                    nc.vector.tensor_scalar(o[:], h[:, nt, :],
                                            mu3[:, i:i + 1],
                                            r[:, i:i + 1],
                                            mybir.AluOpType.subtract,
                                            mybir.AluOpType.mult)
                    nc.sync.dma_start(out=out[nt], in_=o[:])
    nc.compile()
    return nc


def _prep_weights(W, n_pad, dtype):
    """sign(W) [N,K] -> [N/P, P, K/P, P] tiles of sign(W).T, cast to dtype."""
    N, K = W.shape
    S = np.sign(W).astype(np.float32)
    if n_pad != N:
        S = np.concatenate([S, np.zeros((n_pad - N, K), np.float32)], axis=0)
    # element [nt, p, c, m] = S.T[c*P+p, nt*P+m] = S[nt*P+m, c*P+p]
    A = S.reshape(n_pad // P, P, K // P, P)            # [nt, m, c, p]
    A = np.ascontiguousarray(A.transpose(0, 3, 2, 1))  # [nt, p, c, m]
    return A.astype(dtype)


def _host_reference(inputs):
    """Exact-semantics fallback (general g/b); matches jnp reference to ~1e-7."""
    x = inputs["x"].astype(np.float64)
    h = x
    for i in range(4):
        S = np.sign(inputs[f"W{i}"]).astype(np.float64)
        h = h @ S.T
        mu = h.mean(0)
        var = h.var(0)
        h = inputs[f"g{i}"] * (h - mu) / np.sqrt(var + EPS) + inputs[f"b{i}"]
        if i < 3:
            h = np.sign(h)
    return h.astype(np.float32)


def _prepare(inputs):
    x = inputs["x"]
    xm = x.astype(np.float64)
    xc = xm - xm.mean(0)
    FIX = np.rint(xc * SCALE).astype(np.int64)        # [BATCH, K0]
    d0 = ((FIX + 256) % 512) - 256
    cur = (FIX - d0) // 512
    d1 = ((cur + 256) % 512) - 256
    d2 = (cur - d1) // 512
    if np.abs(d2).max() > 2047:                        # fp16 exact-int bound
        return None
    planes = []
    for dj in (d0, d1, d2):
        # [BATCH, K0] -> per-core [P, K0/P, B] fp16, k-major
        t = dj.astype(np.float16).T                    # [K0, BATCH]
        t = t.reshape(K0 // P, P, BATCH)               # [c, p, batch]
        planes.append(np.ascontiguousarray(t.transpose(1, 0, 2)))  # [p, c, b]
    f8 = ml_dtypes.float8_e4m3
    w0 = _prep_weights(inputs["W0"], N0, np.float16)
    w1 = _prep_weights(inputs["W1"], N1, f8)
    w2 = _prep_weights(inputs["W2"], N2, f8)
    w3 = _prep_weights(inputs["W3"], N3, f8)
    in_maps = []
    for c in range(NCORES):
        sl = slice(c * B, (c + 1) * B)
        in_maps.append({
            "xd0": np.ascontiguousarray(planes[0][:, :, sl]),
            "xd1": np.ascontiguousarray(planes[1][:, :, sl]),
            "xd2": np.ascontiguousarray(planes[2][:, :, sl]),
            "w0": w0, "w1": w1, "w2": w2, "w3": w3,
        })
    return in_maps


def _assemble(results):
    outs = []
    for c in range(NCORES):
        o = results[c]["out"].reshape(N3, B)           # [1024, 512]
        outs.append(o[:1000].T)                        # [512, 1000]
    return np.ascontiguousarray(np.concatenate(outs, axis=0).astype(np.float32))


def _shapes_ok(inputs):
    try:
        if inputs["x"].shape != (BATCH, K0):
            return False
        for i, (n, k) in enumerate(((N0, K0), (N1, K1), (N2, K2), (1000, K3))):
            if inputs[f"W{i}"].shape != (n, k):
                return False
    except Exception:
        return False
    return True


def run(inputs, trace=False):
    inputs = {k: np.asarray(v) for k, v in inputs.items()}
    generic = not _shapes_ok(inputs)
    for i in range(4):
        if not (np.all(inputs[f"g{i}"] == 1.0) and np.all(inputs[f"b{i}"] == 0.0)):
            generic = True
    if generic:
        return _host_reference(inputs), None
    in_maps = _prepare(inputs)
    if in_maps is None:
        return _host_reference(inputs), None
    try:
        if "nc" not in _CACHE:
            _CACHE["nc"] = _build_nc()
        res = run_bass_kernel_spmd(_CACHE["nc"], in_maps,
                                   core_ids=list(range(NCORES)), trace=trace)
        return _assemble(res.results), res
    except Exception as e:  # device/toolchain unavailable: host fallback
        import sys
        print(f"kernel: device path failed ({type(e).__name__}: {e}); "
              "falling back to host reference", file=sys.stderr)
        return _host_reference(inputs), None


def kernel(**inputs):
    out, _ = run(inputs)
    return out


if __name__ == "__main__":
    d = np.load("/root/problem/inputs.npz")
    inputs = {k: d[k] for k in d.files}
    out, res = run(inputs, trace=False)
    ref = np.load("/root/problem/ref_neuron.npy")
    rel = np.linalg.norm((out - ref).astype(np.float64)) / np.linalg.norm(
        ref.astype(np.float64))
    print("Relative error:", rel)
